# revision 2
# baseline (speedup 1.0000x reference)
"""Trainium2 Bass kernel for nn_AttentionModule (B=4, C=64, L=4096).

Reference computation:
    q = wq @ x + bq ; k = wk @ x + bk ; v = wv @ x + bv      (per batch, [C, L])
    attn = softmax(q^T k / sqrt(C), axis=j)
    out = (attn @ v^T)^T + x                                  ([C, L])

Sharding: 8 cores = 4 batches x 2 query-row halves. Each core holds full
K/V for its batch and computes 2048 of the 4096 query rows. No collectives.

Per-core device algorithm (all matmuls bf16, psum f32):
  - projections via augmented matmuls: host passes [wq.T/sqrt(C); bq/sqrt(C)]
    etc. stacked with x augmented by a ones row, so bias adds are free.
  - scores computed transposed: sT[j, i] = sum_c k[c,j] q[c,i] with the
    K-tile stationary -> psum [128 j, 512 i]; exp on ScalarE -> bf16.
  - V is computed in [l, c] layout with a ones column appended (via an
    extra unit column in the weight), so the AV matmul accumulates
    out[c', i] for c'=0..63 AND the softmax denominator in row 64.
  - normalize: reciprocal (DVE) -> partition_broadcast (GPSIMD) ->
    multiply + residual add (DVE) -> DMA out.
"""

import numpy as np
import ml_dtypes

import concourse.bacc as bacc
import concourse.bass as bass
import concourse.mybir as mybir
import concourse.tile as tile
from concourse.bass_utils import run_bass_kernel_spmd

BF16 = ml_dtypes.bfloat16

B, C, L = 4, 64, 4096
LQ = L // 2          # query rows per core
N_CORES = 8
NB = 512             # matmul moving width / psum bank width (f32)
N_IB = LQ // NB      # 4 query blocks per core
N_JT = L // 128      # 32 key tiles of 128
CHUNK = 2            # j-tiles per exp chunk (exp reads [128, 1024] across 2 banks)
N_CH = N_JT // CHUNK  # 16 chunks per query block

FP32 = mybir.dt.float32
BF = mybir.dt.bfloat16

TRACE = False
LAST_RESULT = None

_cache = {}


def _build():
    nc = bacc.Bacc("TRN2", target_bir_lowering=False, debug=False)

    xa_d = nc.dram_tensor("xa", [C + 1, L], BF, kind="ExternalInput")
    xq_d = nc.dram_tensor("xq", [C + 1, LQ], BF, kind="ExternalInput")
    xr_d = nc.dram_tensor("xr", [C, LQ], FP32, kind="ExternalInput")
    wq_d = nc.dram_tensor("wq", [C + 1, C], BF, kind="ExternalInput")
    wk_d = nc.dram_tensor("wk", [C + 1, C], BF, kind="ExternalInput")
    wv_d = nc.dram_tensor("wv", [C + 1, C + 1], BF, kind="ExternalInput")
    out_d = nc.dram_tensor("out", [C, LQ], FP32, kind="ExternalOutput")

    with tile.TileContext(nc) as tc:
        with (
            tc.tile_pool(name="const", bufs=1) as cpool,
            tc.tile_pool(name="work", bufs=1) as wpool,
            tc.tile_pool(name="expt", bufs=6) as epool,
            tc.tile_pool(name="norm", bufs=3) as npool,
            tc.tile_pool(name="spsum", bufs=2, space="PSUM") as spool,
            tc.tile_pool(name="opsum", bufs=2, space="PSUM") as opool,
        ):
            # ---- load inputs ----
            x_sb = cpool.tile([C + 1, 4, 1024], BF)      # xa in 4 column chunks
            xq_sb = cpool.tile([C + 1, 2, 1024], BF)
            xr_sb = cpool.tile([C, LQ], FP32)
            wq_sb = cpool.tile([C + 1, C], BF)
            wk_sb = cpool.tile([C + 1, C], BF)
            wv_sb = cpool.tile([C + 1, C + 1], BF)
            for c in range(4):
                nc.sync.dma_start(x_sb[:, c, :], xa_d[:, c * 1024:(c + 1) * 1024])
            for c in range(2):
                nc.sync.dma_start(xq_sb[:, c, :], xq_d[:, c * 1024:(c + 1) * 1024])
            nc.sync.dma_start(wq_sb[:], wq_d[:])
            nc.sync.dma_start(wk_sb[:], wk_d[:])
            nc.sync.dma_start(wv_sb[:], wv_d[:])
            nc.sync.dma_start(xr_sb[:], xr_d[:])

            q_sb = wpool.tile([C, LQ], BF)
            k_sb = wpool.tile([C, L], BF)
            v_sb = wpool.tile([128, N_JT, C + 1], BF)    # [l-tile, jt, c'] layout
            out_sb = wpool.tile([C, LQ], FP32)

            # ---- projections ----
            # q[o, i] for the core's 2048 query columns
            for t in range(LQ // NB):
                ps = spool.tile([C, NB], FP32, tag="s")
                nc.tensor.matmul(
                    ps[:], wq_sb[:], xq_sb[:, t // 2, (t % 2) * NB:(t % 2) * NB + NB],
                    start=True, stop=True)
                nc.vector.tensor_copy(q_sb[:, t * NB:(t + 1) * NB], ps[:])
            # k[o, j] full length
            for t in range(L // NB):
                ps = spool.tile([C, NB], FP32, tag="s")
                nc.tensor.matmul(
                    ps[:], wk_sb[:], x_sb[:, t // 2, (t % 2) * NB:(t % 2) * NB + NB],
                    start=True, stop=True)
                nc.vector.tensor_copy(k_sb[:, t * NB:(t + 1) * NB], ps[:])
            # v in [l, c'] layout, 4 l-tiles of 128 per psum tile
            for t in range(8):
                ps = spool.tile([128, 4 * (C + 1)], FP32, tag="s")
                for u in range(4):
                    l = t * 512 + u * 128
                    nc.tensor.matmul(
                        ps[:, u * (C + 1):(u + 1) * (C + 1)],
                        x_sb[:, l // 1024, (l % 1024):(l % 1024) + 128],
                        wv_sb[:], start=True, stop=True)
                nc.vector.tensor_copy(v_sb[:, t * 4:(t + 1) * 4, :], ps[:])

            # ---- attention: stream 64 chunks (4 blocks x 16 chunks) ----
            o_ps = [None] * N_IB
            pending = []  # one-chunk delay so PE never waits on ACT

            def emit_av(ib, c2, e_t):
                for u in range(CHUNK):
                    jt = c2 * CHUNK + u
                    nc.tensor.matmul(
                        o_ps[ib][:], v_sb[:, jt, :], e_t[:, u * NB:(u + 1) * NB],
                        start=(jt == 0), stop=(jt == N_JT - 1))
                if c2 == N_CH - 1:
                    emit_norm(ib)

            def emit_norm(ib):
                recip = npool.tile([1, NB], FP32, tag="recip")
                nc.vector.reciprocal(recip[:], o_ps[ib][C:C + 1, :])
                bcast = npool.tile([C, NB], FP32, tag="bcast")
                nc.gpsimd.partition_broadcast(bcast[:], recip[:])
                tmp = npool.tile([C, NB], FP32, tag="tmp")
                nc.vector.tensor_mul(tmp[:], o_ps[ib][0:C, :], bcast[:])
                sl = slice(ib * NB, (ib + 1) * NB)
                nc.vector.tensor_add(out_sb[:, sl], tmp[:], xr_sb[:, sl])
                nc.sync.dma_start(out_d[:, sl], out_sb[:, sl])

            for g in range(N_IB * N_CH):
                ib, c2 = g // N_CH, g % N_CH
                if c2 == 0:
                    o_ps[ib] = opool.tile([C + 1, NB], FP32, tag="o", name="o_ps")
                q_blk = q_sb[:, ib * NB:(ib + 1) * NB]
                s_ps = spool.tile([128, CHUNK * NB], FP32, tag="s")
                for u in range(CHUNK):
                    jt = c2 * CHUNK + u
                    nc.tensor.matmul(
                        s_ps[:, u * NB:(u + 1) * NB],
                        k_sb[:, jt * 128:(jt + 1) * 128], q_blk,
                        start=True, stop=True)
                e_t = epool.tile([128, CHUNK * NB], BF, tag="e")
                nc.scalar.activation(e_t[:], s_ps[:], mybir.ActivationFunctionType.Exp)
                pending.append((ib, c2, e_t))
                if len(pending) > 1:
                    emit_av(*pending.pop(0))
            emit_av(*pending.pop(0))

    nc.compile()
    return nc


def _get_nc():
    if "nc" not in _cache:
        _cache["nc"] = _build()
    return _cache["nc"]


def _prep_in_maps(x, wq, bq, wk, bk, wv, bv):
    s = 1.0 / np.sqrt(np.float32(C))
    wqT = np.concatenate([wq.T * s, (bq * s)[None, :]], axis=0).astype(BF16)
    wkT = np.concatenate([wk.T, bk[None, :]], axis=0).astype(BF16)
    wvT = np.zeros((C + 1, C + 1), np.float32)
    wvT[:C, :C] = wv.T
    wvT[C, :C] = bv
    wvT[C, C] = 1.0
    wvT = wvT.astype(BF16)

    in_maps = []
    for core in range(N_CORES):
        b, h = core // 2, core % 2
        xa = np.concatenate([x[b], np.ones((1, L), np.float32)], axis=0).astype(BF16)
        sl = slice(h * LQ, (h + 1) * LQ)
        in_maps.append({
            "xa": xa,
            "xq": np.ascontiguousarray(xa[:, sl]),
            "xr": np.ascontiguousarray(x[b][:, sl].astype(np.float32)),
            "wq": wqT, "wk": wkT, "wv": wvT,
        })
    return in_maps


def kernel(x, wq, bq, wk, bk, wv, bv):
    global LAST_RESULT
    x = np.asarray(x, np.float32)
    wq, bq = np.asarray(wq, np.float32), np.asarray(bq, np.float32)
    wk, bk = np.asarray(wk, np.float32), np.asarray(bk, np.float32)
    wv, bv = np.asarray(wv, np.float32), np.asarray(bv, np.float32)

    nc = _get_nc()
    in_maps = _prep_in_maps(x, wq, bq, wk, bk, wv, bv)
    res = run_bass_kernel_spmd(nc, in_maps, core_ids=list(range(N_CORES)),
                               trace=TRACE)
    LAST_RESULT = res

    out = np.empty((B, C, L), np.float32)
    for core in range(N_CORES):
        b, h = core // 2, core % 2
        out[b, :, h * LQ:(h + 1) * LQ] = res.results[core]["out"]
    return out
